# revision 5
# baseline (speedup 1.0000x reference)
"""AttentionBlock (GroupNorm -> qkv -> full 4096-token attention -> GroupNorm
-> SwiGLU MLP -> residual) on 8 Trainium2 NeuronCores.

Sharding: core = (batch b = core//2, query-token half h = core%2). Each core
computes k/v over all 4096 tokens of its image and attention rows for its
2048 query tokens (host permutes tokens so queries are always columns
0..2047 -> one static SPMD program). The attention is computed fully
transposed (S^T = k^T q with keys on partitions) so no transposes are needed
anywhere.

GroupNorm-1 is folded ENTIRELY into the q/k/v weights on the host
(W' = W.diag(alpha), constants W.beta + bias): the device consumes x as
bf16 directly and launch A starts on TensorE immediately (no affine
pre-pass). The k-bias is dropped on device (a per-query constant added to
every key logit cancels in softmax); the v "bias" (W_v.beta + v_b) is added
on the host after normalization. Softmax row-sums are accumulated on the
Vector engine (running sum of the exp tiles) and reduced over partitions on
the host -- no TensorE ones-matmul. The MLP GroupNorm's cross-core stats are
combined on the host between the two launches, which also folds them into
the MLP weights; SwiGLU's z = silu(g + c1) is a single ScalarE op.
All GEMMs run in bf16 with fp32 PSUM accumulation; exp runs on ScalarE in
fp32 straight out of PSUM; softmax normalization happens on the host in
fp32 during the inter-launch step.
"""
import sys
from contextlib import ExitStack

for _p in ("/opt/trn_rl_repo", "/root/.axon_site/_ro/trn_rl_repo"):
    if _p not in sys.path:
        sys.path.insert(0, _p)

import numpy as np
import ml_dtypes

import concourse.bass as bass
import concourse.tile as tile
from concourse import bacc, mybir, bass_utils

F32 = mybir.dt.float32
BF16 = mybir.dt.bfloat16
F16 = mybir.dt.float16
AF = mybir.ActivationFunctionType
ALU = mybir.AluOpType
BF = ml_dtypes.bfloat16
HF = np.float16

P = 128          # partitions
C = 512          # channels
CT = C // P      # 4 channel tiles (== 4 groups: each group is one c-tile)
HW = 4096        # tokens per image
NT = 2048        # query tokens per core
NI = NT // 512   # i-chunks of 512
NJ = HW // P     # 32 j-tiles of 128
B = 4
EPS = 1e-6
SCALE = C ** -0.5


def build_launch_a(repeat: int = 1):
    nc = bacc.Bacc("TRN2", target_bir_lowering=False, debug=False, num_devices=8)

    x = nc.dram_tensor("x", [C, HW], F16, kind="ExternalInput").ap()
    wqT = nc.dram_tensor("wqT", [C, C], F16, kind="ExternalInput").ap()
    wkT = nc.dram_tensor("wkT", [C, C], F16, kind="ExternalInput").ap()
    wvT = nc.dram_tensor("wvT", [C, C], F16, kind="ExternalInput").ap()
    qc = nc.dram_tensor("qc", [P, CT], F32, kind="ExternalInput").ap()

    out_n = nc.dram_tensor("out_n", [C, NT], F16, kind="ExternalOutput").ap()
    es_d = nc.dram_tensor("es", [P, NI * 512], F32, kind="ExternalOutput").ap()

    with tile.TileContext(nc) as tc, ExitStack() as ctx:
        const = ctx.enter_context(tc.tile_pool(name="const", bufs=1))
        qc_t = const.tile([P, CT], F32)
        nc.sync.dma_start(out=qc_t, in_=qc)

        big = ctx.enter_context(tc.tile_pool(name="big", bufs=1))
        k_sb = big.tile([P, CT, HW], F16)
        q_sb = big.tile([P, CT, NT], F16)
        vt_sb = big.tile([P, NJ, C], F16)
        u_big = big.tile([P, CT, NT], F16)  # unnormalized attention out
        es_sb = big.tile([P, NI, 512], F32)  # per-partition rowsum partials

        for rep in range(repeat):
            # ---- phase 1: load x (bf16, GroupNorm folded into weights) ----
            with tc.tile_pool(name=f"xin_{rep}", bufs=1) as px, \
                 tc.tile_pool(name=f"psA_{rep}", bufs=2, space="PSUM") as psA:
                xb = px.tile([P, CT, HW], F16, name=f"x_{rep}")
                for ct in range(CT):
                    nc.sync.dma_start(out=xb[:, ct, :],
                                      in_=x[ct * P:(ct + 1) * P, :])

                wq_t = px.tile([P, CT, C], F16)
                wk_t = px.tile([P, CT, C], F16)
                wv_t = px.tile([P, CT, C], F16)
                for ci in range(CT):
                    nc.sync.dma_start(out=wq_t[:, ci, :], in_=wqT[ci * P:(ci + 1) * P, :])
                    nc.sync.dma_start(out=wk_t[:, ci, :], in_=wkT[ci * P:(ci + 1) * P, :])
                    nc.sync.dma_start(out=wv_t[:, ci, :], in_=wvT[ci * P:(ci + 1) * P, :])

                # ---- phase 2: q/k/vT GEMMs ----
                for co in range(CT):
                    for jc in range(HW // 512):
                        pk = psA.tile([P, 512], F32, tag="g")
                        for ci in range(CT):
                            nc.tensor.matmul(pk, wk_t[:, ci, co * P:(co + 1) * P],
                                             xb[:, ci, jc * 512:(jc + 1) * 512],
                                             start=(ci == 0), stop=(ci == CT - 1))
                        nc.vector.tensor_copy(
                            out=k_sb[:, co, jc * 512:(jc + 1) * 512], in_=pk)
                    for icc in range(NT // 512):
                        pq = psA.tile([P, 512], F32, tag="g")
                        for ci in range(CT):
                            nc.tensor.matmul(pq, wq_t[:, ci, co * P:(co + 1) * P],
                                             xb[:, ci, icc * 512:(icc + 1) * 512],
                                             start=(ci == 0), stop=(ci == CT - 1))
                        nc.vector.tensor_scalar_add(
                            out=q_sb[:, co, icc * 512:(icc + 1) * 512],
                            in0=pq, scalar1=qc_t[:, co:co + 1])
                # vT: [j, c_out]  (v constant folded into attention output on host)
                for jt in range(NJ):
                    pv = psA.tile([P, C], F32, tag="g")
                    for ci in range(CT):
                        nc.tensor.matmul(pv, xb[:, ci, jt * P:(jt + 1) * P],
                                         wv_t[:, ci, :],
                                         start=(ci == 0), stop=(ci == CT - 1))
                    nc.vector.tensor_copy(out=vt_sb[:, jt, :], in_=pv)

            # ---- phase 3: attention (S^T -> exp -> U, rowsums on DVE) ----
            with tc.tile_pool(name=f"pexp_{rep}", bufs=2) as pexp, \
                 tc.tile_pool(name=f"psS_{rep}", bufs=2, space="PSUM") as psS, \
                 tc.tile_pool(name=f"psU_{rep}", bufs=4, space="PSUM") as psU:
                for ic in range(NI):
                    isl = slice(ic * 512, (ic + 1) * 512)
                    expst = pexp.tile([P, NJ, 512], F16, tag="e", name=f"e_{rep}_{ic}")
                    s_ps = []
                    u_ps = [psU.tile([P, 512], F32, tag="u", name=f"u_{rep}_{ic}_{cc}")
                            for cc in range(CT)]

                    def emit_S(jt):
                        ps = psS.tile([P, 512], F32, tag="S")
                        for ci in range(CT):
                            nc.tensor.matmul(ps, k_sb[:, ci, jt * P:(jt + 1) * P],
                                             q_sb[:, ci, isl],
                                             start=(ci == 0), stop=(ci == CT - 1))
                        s_ps.append(ps)

                    emit_S(0)
                    for jt in range(NJ):
                        if jt + 1 < NJ:
                            emit_S(jt + 1)
                        nc.scalar.activation(out=expst[:, jt, :], in_=s_ps[jt],
                                             func=AF.Exp, scale=SCALE)
                        for cc in range(CT):
                            nc.tensor.matmul(u_ps[cc],
                                             vt_sb[:, jt, cc * P:(cc + 1) * P],
                                             expst[:, jt, :],
                                             start=(jt == 0), stop=(jt == NJ - 1))
                        if jt == 0:
                            nc.vector.tensor_copy(out=es_sb[:, ic, :],
                                                  in_=expst[:, jt, :])
                        else:
                            nc.vector.tensor_tensor(es_sb[:, ic, :],
                                                    es_sb[:, ic, :],
                                                    expst[:, jt, :],
                                                    ALU.add)

                    for cc in range(CT):
                        nc.vector.tensor_copy(out=u_big[:, cc, isl], in_=u_ps[cc])
                        nc.sync.dma_start(out=out_n[cc * P:(cc + 1) * P, isl],
                                          in_=u_big[:, cc, isl])
                    nc.sync.dma_start(out=es_d[:, ic * 512:(ic + 1) * 512],
                                      in_=es_sb[:, ic, :])

    nc.compile()
    return nc


def build_launch_b(repeat: int = 1):
    nc = bacc.Bacc("TRN2", target_bir_lowering=False, debug=False, num_devices=8)

    on = nc.dram_tensor("on", [C, NT], F16, kind="ExternalInput").ap()
    xh = nc.dram_tensor("xh", [C, NT], F16, kind="ExternalInput").ap()
    w1T = nc.dram_tensor("w1T", [C, 2 * C], F16, kind="ExternalInput").ap()
    c1 = nc.dram_tensor("c1", [P, 2 * CT], F32, kind="ExternalInput").ap()
    w2T = nc.dram_tensor("w2T", [C, C], F16, kind="ExternalInput").ap()
    b2 = nc.dram_tensor("b2", [P, CT], F32, kind="ExternalInput").ap()

    y = nc.dram_tensor("y", [C, NT], F32, kind="ExternalOutput").ap()

    with tile.TileContext(nc) as tc, ExitStack() as ctx:
        big = ctx.enter_context(tc.tile_pool(name="big", bufs=1))
        psG = ctx.enter_context(tc.tile_pool(name="psG", bufs=4, space="PSUM"))
        pout = ctx.enter_context(tc.tile_pool(name="pout", bufs=3))

        on_t = big.tile([P, CT, NT], F16)
        w1_t = big.tile([P, CT, 2 * C], F16)
        w2_t = big.tile([P, CT, C], F16)
        c1_t = big.tile([P, 2 * CT], F32)
        b2_t = big.tile([P, CT], F32)
        for ci in range(CT):
            nc.sync.dma_start(out=on_t[:, ci, :], in_=on[ci * P:(ci + 1) * P, :])
            nc.sync.dma_start(out=w1_t[:, ci, :], in_=w1T[ci * P:(ci + 1) * P, :])
            nc.sync.dma_start(out=w2_t[:, ci, :], in_=w2T[ci * P:(ci + 1) * P, :])
        nc.sync.dma_start(out=c1_t, in_=c1)
        nc.sync.dma_start(out=b2_t, in_=b2)

        z_sb = big.tile([P, CT, NT], F32)    # (g + c1g) * sigmoid(g + c1g)
        s_sb = big.tile([P, CT, NT], F32)    # sigmoid(g + c1g)
        h_sb = big.tile([P, CT, NT], F16)   # (a + c1a) * z

        for ic in range(NI * repeat):
            ic = ic % NI
            isl = slice(ic * 512, (ic + 1) * 512)
            # 'g' half: z = (g + c1g) * sigmoid(g + c1g)
            for gt in range(CT):
                pm = psG.tile([P, 512], F32, tag="m1")
                for ci in range(CT):
                    nc.tensor.matmul(pm, w1_t[:, ci, (CT + gt) * P:(CT + gt + 1) * P],
                                     on_t[:, ci, isl],
                                     start=(ci == 0), stop=(ci == CT - 1))
                nc.scalar.activation(out=s_sb[:, gt, isl], in_=pm,
                                     func=AF.Sigmoid,
                                     bias=c1_t[:, CT + gt:CT + gt + 1], scale=1.0)
                nc.vector.scalar_tensor_tensor(out=z_sb[:, gt, isl], in0=pm,
                                               scalar=c1_t[:, CT + gt:CT + gt + 1],
                                               in1=s_sb[:, gt, isl],
                                               op0=ALU.add, op1=ALU.mult)
            # 'a' half: h = (a + c1a) * z, fused from PSUM
            for ot in range(CT):
                pm = psG.tile([P, 512], F32, tag="m1")
                for ci in range(CT):
                    nc.tensor.matmul(pm, w1_t[:, ci, ot * P:(ot + 1) * P],
                                     on_t[:, ci, isl],
                                     start=(ci == 0), stop=(ci == CT - 1))
                nc.vector.scalar_tensor_tensor(out=h_sb[:, ot, isl], in0=pm,
                                               scalar=c1_t[:, ot:ot + 1],
                                               in1=z_sb[:, ot, isl],
                                               op0=ALU.add, op1=ALU.mult)
            for ot in range(CT):
                pm2 = psG.tile([P, 512], F32, tag="m2")
                for cc in range(CT):
                    nc.tensor.matmul(pm2, w2_t[:, cc, ot * P:(ot + 1) * P],
                                     h_sb[:, cc, isl],
                                     start=(cc == 0), stop=(cc == CT - 1))
                xt = pout.tile([P, 512], F16, tag="xt")
                nc.sync.dma_start(out=xt, in_=xh[ot * P:(ot + 1) * P, isl])
                yt = pout.tile([P, 512], F32, tag="yt")
                nc.vector.scalar_tensor_tensor(out=yt, in0=pm2,
                                               scalar=b2_t[:, ot:ot + 1],
                                               in1=xt, op0=ALU.add, op1=ALU.add)
                nc.sync.dma_start(out=y[ot * P:(ot + 1) * P, isl], in_=yt)

    nc.compile()
    return nc


def _tile_vec(v):
    """[C] -> [P, CT] with partition = channel % 128, col = channel // 128."""
    return np.ascontiguousarray(np.asarray(v, np.float32).reshape(-1, P).T)


_CACHE = {}


def _get_ncs():
    if "a" not in _CACHE:
        _CACHE["a"] = build_launch_a()
        _CACHE["b"] = build_launch_b()
    return _CACHE["a"], _CACHE["b"]


def prep_a_inmaps(inputs):
    x = np.asarray(inputs["x"], np.float32).reshape(B, C, HW)
    qwT = np.asarray(inputs["q_w"], np.float32).T
    kwT = np.asarray(inputs["k_w"], np.float32).T
    vwT = np.asarray(inputs["v_w"], np.float32).T
    nsc = np.asarray(inputs["norm_scale"], np.float64)
    nbi = np.asarray(inputs["norm_bias"], np.float64)
    qb = np.asarray(inputs["q_b"], np.float32)

    # GroupNorm-1 stats on host (f64), folded into weights + constants
    per_img = []
    for b in range(B):
        g = x[b].reshape(CT, P * HW).astype(np.float64)
        mean_g = g.mean(axis=1)
        var_g = g.var(axis=1)
        rstd_c = np.repeat(1.0 / np.sqrt(var_g + EPS), P)
        mean_c = np.repeat(mean_g, P)
        alpha = (rstd_c * nsc).astype(np.float32)
        beta = (nbi - mean_c * rstd_c * nsc).astype(np.float32)
        wqs = np.ascontiguousarray(qwT * alpha[:, None]).astype(HF)
        wks = np.ascontiguousarray(kwT * alpha[:, None]).astype(HF)
        wvs = np.ascontiguousarray(vwT * alpha[:, None]).astype(HF)
        qcv = _tile_vec(np.asarray(inputs["q_w"], np.float32) @ beta + qb)
        per_img.append((wqs, wks, wvs, qcv))

    a_maps = []
    for core in range(8):
        b, h = core // 2, core % 2
        xb = x[b]
        xp = xb if h == 0 else np.concatenate([xb[:, NT:], xb[:, :NT]], axis=1)
        wqs, wks, wvs, qcv = per_img[b]
        a_maps.append(dict(x=np.ascontiguousarray(xp).astype(HF),
                           wqT=wqs, wkT=wks, wvT=wvs, qc=qcv))
    return a_maps


def normalize_a_results(inputs, results):
    # ---- host: normalize softmax, add v constant, GroupNorm-2 stats ----
    x = np.asarray(inputs["x"], np.float32).reshape(B, C, HW)
    vw = np.asarray(inputs["v_w"], np.float32)
    vb = np.asarray(inputs["v_b"], np.float32)
    nsc = np.asarray(inputs["norm_scale"], np.float64)
    nbi = np.asarray(inputs["norm_bias"], np.float64)
    vcs = []
    for b in range(B):
        g = x[b].reshape(CT, P * HW).astype(np.float64)
        mean_g = g.mean(axis=1)
        var_g = g.var(axis=1)
        rstd_c = np.repeat(1.0 / np.sqrt(var_g + EPS), P)
        mean_c = np.repeat(mean_g, P)
        beta = (nbi - mean_c * rstd_c * nsc).astype(np.float32)
        vcs.append((vw @ beta + vb).astype(np.float32))

    norm = []
    for core, r in enumerate(results):
        b = core // 2
        U = r["out_n"].astype(np.float32)
        rs = r["es"].astype(np.float64).sum(axis=0).astype(np.float32)
        out = U / rs[None, :] + vcs[b][:, None]
        outb = out.astype(HF)
        of = outb.astype(np.float64).reshape(CT, P, NT)
        pst = np.empty((P, 2 * CT), np.float64)
        pst[:, 0::2] = of.sum(axis=2).T
        pst[:, 1::2] = (of ** 2).sum(axis=2).T
        norm.append(dict(out_n=outb, pstats=pst))
    return norm


def combine_stats_and_prep_b(inputs, norm):
    x = np.asarray(inputs["x"], np.float32).reshape(B, C, HW)
    w1 = np.asarray(inputs["mlp_w1"], np.float32)
    b1 = np.asarray(inputs["mlp_b1"], np.float32)
    w2 = np.asarray(inputs["mlp_w2"], np.float32)
    msc = np.asarray(inputs["mlp_norm_scale"], np.float32)
    mbi = np.asarray(inputs["mlp_norm_bias"], np.float32)
    w2T = np.ascontiguousarray(w2.T).astype(HF)
    b2t = _tile_vec(inputs["mlp_b2"])

    b_maps = []
    for core in range(8):
        b, h = core // 2, core % 2
        ps = norm[2 * b]["pstats"] + norm[2 * b + 1]["pstats"]
        S = ps[:, 0::2].sum(axis=0)
        SQ = ps[:, 1::2].sum(axis=0)
        N = P * HW
        mean_g = S / N
        var_g = SQ / N - mean_g ** 2
        rstd_g = 1.0 / np.sqrt(var_g + EPS)
        mean_c = np.repeat(mean_g, P)
        rstd_c = np.repeat(rstd_g, P)
        alpha2 = (rstd_c * msc).astype(np.float32)
        beta2 = (mbi - mean_c * rstd_c * msc).astype(np.float32)
        w1Ts = np.ascontiguousarray(w1.T * alpha2[:, None]).astype(HF)
        c1 = (b1 + w1 @ beta2).astype(np.float32)
        c1t = np.ascontiguousarray(c1.reshape(2 * CT, P).T)
        xh = np.ascontiguousarray(x[b][:, h * NT:(h + 1) * NT]).astype(HF)
        b_maps.append(dict(on=norm[core]["out_n"], xh=xh, w1T=w1Ts,
                           c1=c1t, w2T=w2T, b2=b2t))
    return b_maps


def assemble_y(results):
    y = np.empty((B, C, HW), np.float32)
    for core in range(8):
        b, h = core // 2, core % 2
        y[b][:, h * NT:(h + 1) * NT] = results[core]["y"]
    return y.reshape(B, C, 64, 64)


def kernel(**inputs):
    nca, ncb = _get_ncs()
    a_maps = prep_a_inmaps(inputs)
    res_a = bass_utils.run_bass_kernel_spmd(nca, a_maps, core_ids=list(range(8)))
    norm = normalize_a_results(inputs, res_a.results)
    b_maps = combine_stats_and_prep_b(inputs, norm)
    res_b = bass_utils.run_bass_kernel_spmd(ncb, b_maps, core_ids=list(range(8)))
    return assemble_y(res_b.results)


# revision 8
# speedup vs baseline: 1.0388x; 1.0388x over previous
"""AttentionBlock (GroupNorm -> qkv -> full 4096-token attention -> GroupNorm
-> SwiGLU MLP -> residual) on 8 Trainium2 NeuronCores.

Sharding: core = (batch b = core//2, query-token half h = core%2). Each core
computes k/v over all 4096 tokens of its image and attention rows for its
2048 query tokens (host permutes tokens so queries are always columns
0..2047 -> one static SPMD program). The attention is computed fully
transposed (S^T = k^T q with keys on partitions) so no transposes are needed
anywhere.

GroupNorm-1 is folded ENTIRELY into the q/k/v weights on the host
(W' = W.diag(alpha), constants W.beta + bias): the device consumes x as
bf16 directly and launch A starts on TensorE immediately (no affine
pre-pass). The k-bias is dropped on device (a per-query constant added to
every key logit cancels in softmax); the v "bias" (W_v.beta + v_b) is added
on the host after normalization. Softmax row-sums are accumulated on the
Vector engine (running sum of the exp tiles) and reduced over partitions on
the host -- no TensorE ones-matmul. The MLP GroupNorm's cross-core stats are
combined on the host between the two launches, which also folds them into
the MLP weights; SwiGLU's z = silu(g + c1) is a single ScalarE op.
All GEMMs run in bf16 with fp32 PSUM accumulation; exp runs on ScalarE in
fp32 straight out of PSUM; softmax normalization happens on the host in
fp32 during the inter-launch step.
"""
import sys
from contextlib import ExitStack

for _p in ("/opt/trn_rl_repo", "/root/.axon_site/_ro/trn_rl_repo"):
    if _p not in sys.path:
        sys.path.insert(0, _p)

import numpy as np
import ml_dtypes

import concourse.bass as bass
import concourse.tile as tile
from concourse import bacc, mybir, bass_utils

F32 = mybir.dt.float32
BF16 = mybir.dt.bfloat16
F16 = mybir.dt.float16
AF = mybir.ActivationFunctionType
ALU = mybir.AluOpType
BF = ml_dtypes.bfloat16
HF = np.float16

P = 128          # partitions
C = 512          # channels
CT = C // P      # 4 channel tiles (== 4 groups: each group is one c-tile)
HW = 4096        # tokens per image
NT = 2048        # query tokens per core
NI = NT // 512   # i-chunks of 512
NJ = HW // P     # 32 j-tiles of 128
B = 4
EPS = 1e-6
SCALE = C ** -0.5


def build_launch_a(repeat: int = 1):
    nc = bacc.Bacc("TRN2", target_bir_lowering=False, debug=False, num_devices=8)

    x = nc.dram_tensor("x", [C, HW], F16, kind="ExternalInput").ap()
    wqT = nc.dram_tensor("wqT", [C, C], F16, kind="ExternalInput").ap()
    wkT = nc.dram_tensor("wkT", [C, C], F16, kind="ExternalInput").ap()
    wvT = nc.dram_tensor("wvT", [C, C], F16, kind="ExternalInput").ap()
    qc = nc.dram_tensor("qc", [P, CT], F32, kind="ExternalInput").ap()

    out_n = nc.dram_tensor("out_n", [C, NT], F16, kind="ExternalOutput").ap()
    es_d = nc.dram_tensor("es", [P, NI * 512], F32, kind="ExternalOutput").ap()

    with tile.TileContext(nc) as tc, ExitStack() as ctx:
        const = ctx.enter_context(tc.tile_pool(name="const", bufs=1))
        qc_t = const.tile([P, CT], F32)
        nc.sync.dma_start(out=qc_t, in_=qc)

        big = ctx.enter_context(tc.tile_pool(name="big", bufs=1))
        k_sb = big.tile([P, CT, HW], F16)
        q_sb = big.tile([P, CT, NT], F16)
        vt_sb = big.tile([P, NJ, C], F16)
        es_sb = big.tile([P, NI, 512], F32)  # per-partition rowsum partials

        # Weights are identical across reps: load once, keep resident.
        wq_t = big.tile([P, CT, C], F16)
        wk_t = big.tile([P, CT, C], F16)
        wv_t = big.tile([P, CT, C], F16)
        for ci in range(CT):
            nc.sync.dma_start(out=wq_t[:, ci, :], in_=wqT[ci * P:(ci + 1) * P, :])
            nc.sync.dma_start(out=wk_t[:, ci, :], in_=wkT[ci * P:(ci + 1) * P, :])
            nc.sync.dma_start(out=wv_t[:, ci, :], in_=wvT[ci * P:(ci + 1) * P, :])

        # Double-buffered x pool: rep r+1's x DMA overlaps rep r's attention
        # phase instead of stalling TensorE at the rep boundary.
        px = ctx.enter_context(tc.tile_pool(name="px", bufs=min(repeat, 2)))

        for rep in range(repeat):
            # ---- phase 1: load x (fp16, GroupNorm folded into weights) ----
            with tc.tile_pool(name=f"psA_{rep}", bufs=2, space="PSUM") as psA:
                xb = px.tile([P, CT, HW], F16, tag="x", name=f"x_{rep}")
                for ct in range(CT):
                    nc.sync.dma_start(out=xb[:, ct, :],
                                      in_=x[ct * P:(ct + 1) * P, :])

                # ---- phase 2: q/k/vT GEMMs ----
                for co in range(CT):
                    for jc in range(HW // 512):
                        pk = psA.tile([P, 512], F32, tag="g")
                        for ci in range(CT):
                            nc.tensor.matmul(pk, wk_t[:, ci, co * P:(co + 1) * P],
                                             xb[:, ci, jc * 512:(jc + 1) * 512],
                                             start=(ci == 0), stop=(ci == CT - 1))
                        nc.vector.tensor_copy(
                            out=k_sb[:, co, jc * 512:(jc + 1) * 512], in_=pk)
                    for icc in range(NT // 512):
                        pq = psA.tile([P, 512], F32, tag="g")
                        for ci in range(CT):
                            nc.tensor.matmul(pq, wq_t[:, ci, co * P:(co + 1) * P],
                                             xb[:, ci, icc * 512:(icc + 1) * 512],
                                             start=(ci == 0), stop=(ci == CT - 1))
                        nc.vector.tensor_scalar_add(
                            out=q_sb[:, co, icc * 512:(icc + 1) * 512],
                            in0=pq, scalar1=qc_t[:, co:co + 1])
                # vT: [j, c_out]  (v constant folded into attention output on host)
                for jt in range(NJ):
                    pv = psA.tile([P, C], F32, tag="g")
                    for ci in range(CT):
                        nc.tensor.matmul(pv, xb[:, ci, jt * P:(jt + 1) * P],
                                         wv_t[:, ci, :],
                                         start=(ci == 0), stop=(ci == CT - 1))
                    nc.vector.tensor_copy(out=vt_sb[:, jt, :], in_=pv)

            # ---- phase 3: attention (S^T -> exp -> U, rowsums on DVE) ----
            with tc.tile_pool(name=f"pexp_{rep}", bufs=1) as pexp, \
                 tc.tile_pool(name=f"pu_{rep}", bufs=3) as pu, \
                 tc.tile_pool(name=f"psS_{rep}", bufs=2, space="PSUM") as psS, \
                 tc.tile_pool(name=f"psU_{rep}", bufs=4, space="PSUM") as psU:
                for ic in range(NI):
                    isl = slice(ic * 512, (ic + 1) * 512)
                    expst = pexp.tile([P, NJ, 512], F16, tag="e", name=f"e_{rep}_{ic}")
                    s_ps = []
                    u_ps = [psU.tile([P, 512], F32, tag="u", name=f"u_{rep}_{ic}_{cc}")
                            for cc in range(CT)]

                    def emit_S(jt):
                        ps = psS.tile([P, 512], F32, tag="S")
                        for ci in range(CT):
                            nc.tensor.matmul(ps, k_sb[:, ci, jt * P:(jt + 1) * P],
                                             q_sb[:, ci, isl],
                                             start=(ci == 0), stop=(ci == CT - 1))
                        s_ps.append(ps)

                    emit_S(0)
                    for jt in range(NJ):
                        if jt + 1 < NJ:
                            emit_S(jt + 1)
                        nc.scalar.activation(out=expst[:, jt, :], in_=s_ps[jt],
                                             func=AF.Exp, scale=SCALE)
                        for cc in range(CT):
                            nc.tensor.matmul(u_ps[cc],
                                             vt_sb[:, jt, cc * P:(cc + 1) * P],
                                             expst[:, jt, :],
                                             start=(jt == 0), stop=(jt == NJ - 1))
                        if jt == 0:
                            nc.vector.tensor_copy(out=es_sb[:, ic, :],
                                                  in_=expst[:, jt, :])
                        else:
                            nc.vector.tensor_tensor(es_sb[:, ic, :],
                                                    es_sb[:, ic, :],
                                                    expst[:, jt, :],
                                                    ALU.add)

                    for cc in range(CT):
                        ut = pu.tile([P, 512], F16, tag="u")
                        nc.vector.tensor_copy(out=ut, in_=u_ps[cc])
                        nc.sync.dma_start(out=out_n[cc * P:(cc + 1) * P, isl],
                                          in_=ut)
                    nc.sync.dma_start(out=es_d[:, ic * 512:(ic + 1) * 512],
                                      in_=es_sb[:, ic, :])

    nc.compile()
    return nc


def build_launch_b(repeat: int = 1):
    nc = bacc.Bacc("TRN2", target_bir_lowering=False, debug=False, num_devices=8)

    on = nc.dram_tensor("on", [C, NT], F16, kind="ExternalInput").ap()
    xh = nc.dram_tensor("xh", [C, NT], F16, kind="ExternalInput").ap()
    w1T = nc.dram_tensor("w1T", [C, 2 * C], F16, kind="ExternalInput").ap()
    c1 = nc.dram_tensor("c1", [P, 2 * CT], F32, kind="ExternalInput").ap()
    w2T = nc.dram_tensor("w2T", [C, C], F16, kind="ExternalInput").ap()
    b2 = nc.dram_tensor("b2", [P, CT], F32, kind="ExternalInput").ap()

    y = nc.dram_tensor("y", [C, NT], F32, kind="ExternalOutput").ap()

    with tile.TileContext(nc) as tc, ExitStack() as ctx:
        big = ctx.enter_context(tc.tile_pool(name="big", bufs=1))
        psG = ctx.enter_context(tc.tile_pool(name="psG", bufs=4, space="PSUM"))
        pout = ctx.enter_context(tc.tile_pool(name="pout", bufs=3))

        on_t = big.tile([P, CT, NT], F16)
        w1_t = big.tile([P, CT, 2 * C], F16)
        w2_t = big.tile([P, CT, C], F16)
        c1_t = big.tile([P, 2 * CT], F32)
        b2_t = big.tile([P, CT], F32)
        for ci in range(CT):
            nc.sync.dma_start(out=on_t[:, ci, :], in_=on[ci * P:(ci + 1) * P, :])
            nc.sync.dma_start(out=w1_t[:, ci, :], in_=w1T[ci * P:(ci + 1) * P, :])
            nc.sync.dma_start(out=w2_t[:, ci, :], in_=w2T[ci * P:(ci + 1) * P, :])
        nc.sync.dma_start(out=c1_t, in_=c1)
        nc.sync.dma_start(out=b2_t, in_=b2)

        z_sb = big.tile([P, CT, NT], F32)    # (g + c1g) * sigmoid(g + c1g)
        s_sb = big.tile([P, CT, NT], F32)    # sigmoid(g + c1g)
        h_sb = big.tile([P, CT, NT], F16)   # (a + c1a) * z

        for ic in range(NI * repeat):
            ic = ic % NI
            isl = slice(ic * 512, (ic + 1) * 512)
            # 'g' half: z = (g + c1g) * sigmoid(g + c1g)
            for gt in range(CT):
                pm = psG.tile([P, 512], F32, tag="m1")
                for ci in range(CT):
                    nc.tensor.matmul(pm, w1_t[:, ci, (CT + gt) * P:(CT + gt + 1) * P],
                                     on_t[:, ci, isl],
                                     start=(ci == 0), stop=(ci == CT - 1))
                nc.scalar.activation(out=s_sb[:, gt, isl], in_=pm,
                                     func=AF.Sigmoid,
                                     bias=c1_t[:, CT + gt:CT + gt + 1], scale=1.0)
                nc.vector.scalar_tensor_tensor(out=z_sb[:, gt, isl], in0=pm,
                                               scalar=c1_t[:, CT + gt:CT + gt + 1],
                                               in1=s_sb[:, gt, isl],
                                               op0=ALU.add, op1=ALU.mult)
            # 'a' half: h = (a + c1a) * z, fused from PSUM
            for ot in range(CT):
                pm = psG.tile([P, 512], F32, tag="m1")
                for ci in range(CT):
                    nc.tensor.matmul(pm, w1_t[:, ci, ot * P:(ot + 1) * P],
                                     on_t[:, ci, isl],
                                     start=(ci == 0), stop=(ci == CT - 1))
                nc.vector.scalar_tensor_tensor(out=h_sb[:, ot, isl], in0=pm,
                                               scalar=c1_t[:, ot:ot + 1],
                                               in1=z_sb[:, ot, isl],
                                               op0=ALU.add, op1=ALU.mult)
            for ot in range(CT):
                pm2 = psG.tile([P, 512], F32, tag="m2")
                for cc in range(CT):
                    nc.tensor.matmul(pm2, w2_t[:, cc, ot * P:(ot + 1) * P],
                                     h_sb[:, cc, isl],
                                     start=(cc == 0), stop=(cc == CT - 1))
                xt = pout.tile([P, 512], F16, tag="xt")
                nc.sync.dma_start(out=xt, in_=xh[ot * P:(ot + 1) * P, isl])
                yt = pout.tile([P, 512], F32, tag="yt")
                nc.vector.scalar_tensor_tensor(out=yt, in0=pm2,
                                               scalar=b2_t[:, ot:ot + 1],
                                               in1=xt, op0=ALU.add, op1=ALU.add)
                nc.sync.dma_start(out=y[ot * P:(ot + 1) * P, isl], in_=yt)

    nc.compile()
    return nc


def _tile_vec(v):
    """[C] -> [P, CT] with partition = channel % 128, col = channel // 128."""
    return np.ascontiguousarray(np.asarray(v, np.float32).reshape(-1, P).T)


_CACHE = {}


def _get_ncs():
    if "a" not in _CACHE:
        _CACHE["a"] = build_launch_a()
        _CACHE["b"] = build_launch_b()
    return _CACHE["a"], _CACHE["b"]


def prep_a_inmaps(inputs):
    x = np.asarray(inputs["x"], np.float32).reshape(B, C, HW)
    qwT = np.asarray(inputs["q_w"], np.float32).T
    kwT = np.asarray(inputs["k_w"], np.float32).T
    vwT = np.asarray(inputs["v_w"], np.float32).T
    nsc = np.asarray(inputs["norm_scale"], np.float64)
    nbi = np.asarray(inputs["norm_bias"], np.float64)
    qb = np.asarray(inputs["q_b"], np.float32)

    # GroupNorm-1 stats on host (f64), folded into weights + constants
    per_img = []
    for b in range(B):
        g = x[b].reshape(CT, P * HW).astype(np.float64)
        mean_g = g.mean(axis=1)
        var_g = g.var(axis=1)
        rstd_c = np.repeat(1.0 / np.sqrt(var_g + EPS), P)
        mean_c = np.repeat(mean_g, P)
        alpha = (rstd_c * nsc).astype(np.float32)
        beta = (nbi - mean_c * rstd_c * nsc).astype(np.float32)
        wqs = np.ascontiguousarray(qwT * alpha[:, None]).astype(HF)
        wks = np.ascontiguousarray(kwT * alpha[:, None]).astype(HF)
        wvs = np.ascontiguousarray(vwT * alpha[:, None]).astype(HF)
        qcv = _tile_vec(np.asarray(inputs["q_w"], np.float32) @ beta + qb)
        per_img.append((wqs, wks, wvs, qcv))

    a_maps = []
    for core in range(8):
        b, h = core // 2, core % 2
        xb = x[b]
        xp = xb if h == 0 else np.concatenate([xb[:, NT:], xb[:, :NT]], axis=1)
        wqs, wks, wvs, qcv = per_img[b]
        a_maps.append(dict(x=np.ascontiguousarray(xp).astype(HF),
                           wqT=wqs, wkT=wks, wvT=wvs, qc=qcv))
    return a_maps


def normalize_a_results(inputs, results):
    # ---- host: normalize softmax, add v constant, GroupNorm-2 stats ----
    x = np.asarray(inputs["x"], np.float32).reshape(B, C, HW)
    vw = np.asarray(inputs["v_w"], np.float32)
    vb = np.asarray(inputs["v_b"], np.float32)
    nsc = np.asarray(inputs["norm_scale"], np.float64)
    nbi = np.asarray(inputs["norm_bias"], np.float64)
    vcs = []
    for b in range(B):
        g = x[b].reshape(CT, P * HW).astype(np.float64)
        mean_g = g.mean(axis=1)
        var_g = g.var(axis=1)
        rstd_c = np.repeat(1.0 / np.sqrt(var_g + EPS), P)
        mean_c = np.repeat(mean_g, P)
        beta = (nbi - mean_c * rstd_c * nsc).astype(np.float32)
        vcs.append((vw @ beta + vb).astype(np.float32))

    norm = []
    for core, r in enumerate(results):
        b = core // 2
        U = r["out_n"].astype(np.float32)
        rs = r["es"].astype(np.float64).sum(axis=0).astype(np.float32)
        out = U / rs[None, :] + vcs[b][:, None]
        outb = out.astype(HF)
        of = outb.astype(np.float64).reshape(CT, P, NT)
        pst = np.empty((P, 2 * CT), np.float64)
        pst[:, 0::2] = of.sum(axis=2).T
        pst[:, 1::2] = (of ** 2).sum(axis=2).T
        norm.append(dict(out_n=outb, pstats=pst))
    return norm


def combine_stats_and_prep_b(inputs, norm):
    x = np.asarray(inputs["x"], np.float32).reshape(B, C, HW)
    w1 = np.asarray(inputs["mlp_w1"], np.float32)
    b1 = np.asarray(inputs["mlp_b1"], np.float32)
    w2 = np.asarray(inputs["mlp_w2"], np.float32)
    msc = np.asarray(inputs["mlp_norm_scale"], np.float32)
    mbi = np.asarray(inputs["mlp_norm_bias"], np.float32)
    w2T = np.ascontiguousarray(w2.T).astype(HF)
    b2t = _tile_vec(inputs["mlp_b2"])

    b_maps = []
    for core in range(8):
        b, h = core // 2, core % 2
        ps = norm[2 * b]["pstats"] + norm[2 * b + 1]["pstats"]
        S = ps[:, 0::2].sum(axis=0)
        SQ = ps[:, 1::2].sum(axis=0)
        N = P * HW
        mean_g = S / N
        var_g = SQ / N - mean_g ** 2
        rstd_g = 1.0 / np.sqrt(var_g + EPS)
        mean_c = np.repeat(mean_g, P)
        rstd_c = np.repeat(rstd_g, P)
        alpha2 = (rstd_c * msc).astype(np.float32)
        beta2 = (mbi - mean_c * rstd_c * msc).astype(np.float32)
        w1Ts = np.ascontiguousarray(w1.T * alpha2[:, None]).astype(HF)
        c1 = (b1 + w1 @ beta2).astype(np.float32)
        c1t = np.ascontiguousarray(c1.reshape(2 * CT, P).T)
        xh = np.ascontiguousarray(x[b][:, h * NT:(h + 1) * NT]).astype(HF)
        b_maps.append(dict(on=norm[core]["out_n"], xh=xh, w1T=w1Ts,
                           c1=c1t, w2T=w2T, b2=b2t))
    return b_maps


def assemble_y(results):
    y = np.empty((B, C, HW), np.float32)
    for core in range(8):
        b, h = core // 2, core % 2
        y[b][:, h * NT:(h + 1) * NT] = results[core]["y"]
    return y.reshape(B, C, 64, 64)


def kernel(**inputs):
    nca, ncb = _get_ncs()
    a_maps = prep_a_inmaps(inputs)
    res_a = bass_utils.run_bass_kernel_spmd(nca, a_maps, core_ids=list(range(8)))
    norm = normalize_a_results(inputs, res_a.results)
    b_maps = combine_stats_and_prep_b(inputs, norm)
    res_b = bass_utils.run_bass_kernel_spmd(ncb, b_maps, core_ids=list(range(8)))
    return assemble_y(res_b.results)
